# revision 4
# baseline (speedup 1.0000x reference)
"""BRPConvEmbedding (3-layer GraphConv + AvgPool readout) on 8 Trainium2 cores.

v2: super-based layout in bf16 with overlapped AllGathers.

Sharding: graphs split contiguously across cores (32/core); each core owns its
graphs' nodes. Nodes are pre-assigned a row-half bit, then packed per
(core, half) into supers of 256 nodes whose in-edges, split by the source
node's half bit, fit CHT chunks of 128 edge slots per (super, src-half).

Per layer: the per-edge source rows are fetched with one SWDGE dma_gather per
(super, src-half) from a bf16 node table (layer 0's table is expanded on the
host and streamed with HWDGE instead of gathered); the per-edge one-hot
[slots, 256] is built on the DVE (iota + is_equal); the segment-sum runs on
TensorE (lhsT=gathered chunk, rhs=one-hot, PSUM accumulation over chunks);
then agg.T @ W + fused epilogue. Node tables for layers 1-2 are produced by
two bf16 AllGathers per layer (one per row-half), fired as soon as the
producing half's supers finish so they overlap the remaining compute; the
gathers read the AllGather outputs directly (no repack copy).
"""
import numpy as np
from contextlib import ExitStack

import concourse.bacc as bacc
import concourse.mybir as mybir
from concourse import tile
from concourse.bass_utils import run_bass_kernel_spmd

BF16 = mybir.dt.np(mybir.dt.bfloat16)

N_NODES = 50000
N_EDGES = 800000
D = 128
N_LAYERS = 3
N_GRAPHS = 256
NCORES = 8
SSZ = 256                 # dst nodes per super
GPC = N_GRAPHS // NCORES  # graphs per core


# ----------------------------------------------------------------- host prep
def _pack_supers(dA, dB, cap_e):
    """Greedy best-fit-decreasing packing of nodes into supers of <= SSZ nodes
    with sum(dA) <= cap_e and sum(dB) <= cap_e. Returns super id per node."""
    order = np.argsort(-(dA + dB), kind="stable")
    used_n, used_a, used_b = [], [], []
    assign = np.empty(len(dA), dtype=np.int64)
    for i in order:
        a, b = dA[i], dB[i]
        best, best_fit = -1, -1.0
        for j in range(len(used_n)):
            if used_n[j] < SSZ and used_a[j] + a <= cap_e and used_b[j] + b <= cap_e:
                fit = max((used_a[j] + a) / cap_e, (used_b[j] + b) / cap_e)
                if fit > best_fit:
                    best, best_fit = j, fit
        if best < 0:
            used_n.append(0), used_a.append(0), used_b.append(0)
            best = len(used_n) - 1
        assign[i] = best
        used_n[best] += 1
        used_a[best] += a
        used_b[best] += b
    return assign, len(used_n)


def preprocess(feats, W, b, src, dst, graph_ids):
    src = np.asarray(src).astype(np.int64)
    dst = np.asarray(dst).astype(np.int64)
    graph_ids = np.asarray(graph_ids).astype(np.int64)
    feats = np.asarray(feats, dtype=np.float32)

    deg_out = np.maximum(np.bincount(src, minlength=N_NODES), 1).astype(np.float32)
    deg_in = np.maximum(np.bincount(dst, minlength=N_NODES), 1).astype(np.float32)
    norm_out = 1.0 / np.sqrt(deg_out)
    norm_in = 1.0 / np.sqrt(deg_in)

    node_core = graph_ids // GPC

    # ---- row-half assignment: per core, alternate by descending in-degree
    half = np.zeros(N_NODES, dtype=np.int64)
    core_nodes = []
    for c in range(NCORES):
        n = np.nonzero(node_core == c)[0]
        core_nodes.append(n)
        order = np.argsort(-deg_in[n], kind="stable")
        half[n[order[0::2]]] = 0
        half[n[order[1::2]]] = 1

    # ---- per (core, half) super packing over candidate CHT values
    src_half = half[src]
    dA = np.bincount(dst[src_half == 0], minlength=N_NODES)
    dB = np.bincount(dst[src_half == 1], minlength=N_NODES)

    best = None
    for CHT in (15, 16, 17, 18):
        packs, ns_max, ok = {}, 0, True
        for c in range(NCORES):
            for H in (0, 1):
                n = core_nodes[c][half[core_nodes[c]] == H]
                assign, ns = _pack_supers(dA[n], dB[n], CHT * 128)
                packs[(c, H)] = (n, assign)
                ns_max = max(ns_max, ns)
        if 8 * ns_max * SSZ > 32767:
            continue
        slots = ns_max * 2 * CHT
        if best is None or slots < best[0]:
            best = (slots, CHT, ns_max, packs)
    assert best is not None
    _, CHT, NSUP_H, packs = best
    NSUP = 2 * NSUP_H
    HSH = NSUP_H * SSZ        # rows per (core, half)
    P = 2 * NSUP              # pairs (128-row tiles) per core
    RT = NCORES * HSH         # table rows per half
    NI = CHT * 128            # gather slots per (super, src-half)

    # ---- node -> row
    row_local = np.full(N_NODES, -1, dtype=np.int64)   # row within core shard
    for c in range(NCORES):
        for H in (0, 1):
            n, assign = packs[(c, H)]
            order = np.lexsort((n, assign))
            n_s, a_s = n[order], assign[order]
            slot = np.zeros(len(n), dtype=np.int64)
            _, starts = np.unique(a_s, return_index=True)
            for s0, s1 in zip(starts, list(starts[1:]) + [len(n)]):
                slot[s0:s1] = np.arange(s1 - s0)
            row_local[n_s] = (H * NSUP_H + a_s) * SSZ + slot

    # row within the half-table: [core][rows-of-half]
    srow_g = node_core * HSH + (row_local - half * HSH)
    assert srow_g.max() < RT <= 32767

    hn0 = feats * norm_out[:, None]

    # ---- per-core edge layout + tensors
    e_core = node_core[dst]
    e_super = row_local[dst] // SSZ           # global super (0..NSUP-1)
    e_q = src_half                            # src half
    e_dslot = row_local[dst] % SSZ
    e_srow = srow_g[src]

    per_core = []
    for c in range(NCORES):
        m = np.nonzero(e_core == c)[0]
        t = e_super[m] * 2 + e_q[m]
        sr = e_srow[m]
        dslt = e_dslot[m]
        order = np.lexsort((sr, t))
        t, sr, dslt = t[order], sr[order], dslt[order]
        # rank within t
        rank = np.arange(len(m)) - np.searchsorted(t, t, side="left")
        assert rank.max() < NI, f"cap exceeded: {rank.max()} >= {NI}"
        j = rank                                # slot within (super, half)

        # idx array [2*NSUP, 16, NI//16] int16; pads gather row 0 (one-hot
        # zeroes their contribution) -- all-pad or few-pad calls with -1
        # trailing-skip can leave SDMA engines without descriptors and hang
        idx16 = np.zeros((2 * NSUP, 16, NI // 16), dtype=np.int16)
        idx16[t, j % 16, j // 16] = sr.astype(np.int16)
        idx_all = np.tile(idx16, (1, 8, 1)).reshape(2 * NSUP, 128, NI // 16)
        idx_2d = np.ascontiguousarray(
            idx_all.transpose(1, 0, 2).reshape(128, 2 * NSUP * (NI // 16)))

        # dst one-hot scalars [128, 2*NSUP*CHT] bf16, -1 for pad slots
        dstv = np.full((128, 2 * NSUP * CHT), -1.0, dtype=np.float32)
        dstv[j % 128, t * CHT + j // 128] = dslt.astype(np.float32)

        # layer-0 expanded gather stream [128, 2*NSUP*CHT, D] bf16
        t0exp = np.zeros((128, 2 * NSUP * CHT, D), dtype=np.float32)
        t0exp[j % 128, t * CHT + j // 128, :] = hn0[src[m][order]]

        # per-pair node scalars [128, P]
        nodes_c = core_nodes[c]
        lr = row_local[nodes_c]
        ni_t = np.ones((128, P), dtype=np.float32)
        no_t = np.ones((128, P), dtype=np.float32)
        gid_t = np.full((128, P), -1.0, dtype=np.float32)
        ni_t[lr % 128, lr // 128] = norm_in[nodes_c]
        no_t[lr % 128, lr // 128] = norm_out[nodes_c]
        gid_t[lr % 128, lr // 128] = (graph_ids[nodes_c] - c * GPC).astype(np.float32)

        rc = (1.0 / np.maximum(
            np.bincount(graph_ids[nodes_c] - c * GPC, minlength=GPC), 1
        ).astype(np.float32)).reshape(GPC, 1)

        per_core.append(dict(
            idx=idx_2d, dstv=dstv.astype(BF16), t0exp=t0exp.astype(BF16),
            ni=ni_t, no=no_t, gid=gid_t.astype(BF16), rc=rc,
        ))

    shared = dict(
        W=np.ascontiguousarray(
            np.asarray(W, dtype=np.float32).transpose(1, 0, 2)).astype(BF16),
        b_rep=np.broadcast_to(
            np.asarray(b, dtype=np.float32)[None, :, :], (128, N_LAYERS, D)).copy(),
    )
    meta = dict(CHT=CHT, NSUP_H=NSUP_H, NSUP=NSUP, HSH=HSH, P=P, RT=RT, NI=NI)
    return per_core, shared, meta


# ------------------------------------------------------------- device build
def build(meta, rep=1, no_coll=False, no_gather=False, no_stream=False):
    CHT, NSUP_H, NSUP = meta["CHT"], meta["NSUP_H"], meta["NSUP"]
    HSH, P, RT, NI = meta["HSH"], meta["P"], meta["RT"], meta["NI"]
    f32 = mybir.dt.float32
    bf16 = mybir.dt.bfloat16
    IC = NI // 16            # idx cols per (super, half)

    nc = bacc.Bacc("TRN2", target_bir_lowering=False, debug=False,
                   num_devices=NCORES, dynamic_dma_scratch_size=32768,
                   num_swdge_queues=4)

    idx_t = nc.dram_tensor("idx", [128, 2 * NSUP * IC], mybir.dt.int16, kind="ExternalInput")
    dstv_t = nc.dram_tensor("dstv", [128, 2 * NSUP * CHT], bf16, kind="ExternalInput")
    t0exp_t = nc.dram_tensor("t0exp", [128, 2 * NSUP * CHT, D], bf16, kind="ExternalInput")
    ni_t = nc.dram_tensor("ni", [128, P], f32, kind="ExternalInput")
    no_t = nc.dram_tensor("no", [128, P], f32, kind="ExternalInput")
    gid_t = nc.dram_tensor("gid", [128, P], bf16, kind="ExternalInput")
    rc_t = nc.dram_tensor("rc", [GPC, 1], f32, kind="ExternalInput")
    W_t = nc.dram_tensor("W", [128, N_LAYERS, D], bf16, kind="ExternalInput")
    brep_t = nc.dram_tensor("b_rep", [128, N_LAYERS, D], f32, kind="ExternalInput")
    out_t = nc.dram_tensor("out", [GPC, D], f32, kind="ExternalOutput")

    # AllGather outputs: the layer-(l+1) gather tables, one per src-half
    ag_out = [[nc.dram_tensor(f"agout{l}_{q}", [RT, D], bf16,
                              kind="Internal", addr_space="Shared")
               for q in (0, 1)] for l in range(N_LAYERS - 1)]

    with tile.TileContext(nc) as tc, ExitStack() as ctx:
        dram = ctx.enter_context(tc.tile_pool(name="dram", bufs=1, space="DRAM"))
        stat = ctx.enter_context(tc.tile_pool(name="stat", bufs=1))
        gpool = ctx.enter_context(tc.tile_pool(name="gath", bufs=5))
        opool = ctx.enter_context(tc.tile_pool(name="oh", bufs=3))
        spool = ctx.enter_context(tc.tile_pool(name="sb", bufs=6))
        ppool = ctx.enter_context(tc.tile_pool(name="agg_ps", bufs=4, space="PSUM"))
        hpool = ctx.enter_context(tc.tile_pool(name="h_ps", bufs=3, space="PSUM"))
        plpool = ctx.enter_context(tc.tile_pool(name="pool_ps", bufs=1, space="PSUM"))

        # AllGather inputs (per layer, per half)
        hn_half = [[dram.tile([HSH, D], bf16, name=f"hn_half{l}_{q}")
                    for q in (0, 1)] for l in range(N_LAYERS - 1)]

        # ---- statics
        idx_sb = stat.tile([128, 2 * NSUP * IC], mybir.dt.int16)
        nc.sync.dma_start(idx_sb[:], idx_t.ap())
        dstv_sb = stat.tile([128, 2 * NSUP * CHT], bf16)
        nc.sync.dma_start(dstv_sb[:], dstv_t.ap())
        W_sb = stat.tile([128, N_LAYERS, D], bf16)
        nc.sync.dma_start(W_sb[:], W_t.ap())
        brep_sb = stat.tile([128, N_LAYERS, D], f32)
        nc.sync.dma_start(brep_sb[:], brep_t.ap())
        ni_sb = stat.tile([128, P], f32)
        nc.sync.dma_start(ni_sb[:], ni_t.ap())
        no_sb = stat.tile([128, P], f32)
        nc.sync.dma_start(no_sb[:], no_t.ap())
        gid_sb = stat.tile([128, P], bf16)
        nc.sync.dma_start(gid_sb[:], gid_t.ap())
        rc_sb = stat.tile([GPC, 1], f32)
        nc.sync.dma_start(rc_sb[:], rc_t.ap())

        iota16 = stat.tile([128, SSZ], mybir.dt.int16)
        nc.gpsimd.iota(iota16[:], pattern=[[1, SSZ]], base=0, channel_multiplier=0)
        iota_b = stat.tile([128, SSZ], bf16)
        nc.vector.tensor_copy(iota_b[:], iota16[:])

        # graph one-hot [128, P, GPC]
        groh = stat.tile([128, P, GPC], bf16)
        nc.vector.tensor_tensor(
            out=groh[:],
            in0=iota_b[:, :GPC].unsqueeze(1).broadcast_to([128, P, GPC]),
            in1=gid_sb[:].unsqueeze(2).broadcast_to([128, P, GPC]),
            op=mybir.AluOpType.is_equal,
        )

        for _ in range(rep):
            pool_ps = plpool.tile([GPC, D], f32)
            for l in range(N_LAYERS):
                for s in range(NSUP):
                    H = s // NSUP_H
                    g_t = [None, None]
                    oh_t = [None, None]
                    for q in (0, 1):
                        t = s * 2 + q
                        g_t[q] = gpool.tile([128, CHT, D], bf16, tag=f"g{q}", name=f"g{q}")
                        if l == 0 or no_gather:
                            if not no_stream:
                                nc.sync.dma_start(
                                    g_t[q][:], t0exp_t.ap()[:, t * CHT:(t + 1) * CHT, :])
                        else:
                            # num_idxs > 1024 is broken in the gather ucode;
                            # split into <=8-chunk pieces
                            for i, c0 in enumerate(range(0, CHT, 8)):
                                c1 = min(CHT, c0 + 8)
                                nc.gpsimd.dma_gather(
                                    out_ap=g_t[q][:, c0:c1, :],
                                    in_ap=ag_out[l - 1][q].ap(),
                                    idxs_ap=idx_sb[:, t * IC + c0 * 8:
                                                   t * IC + c1 * 8],
                                    num_idxs=(c1 - c0) * 128,
                                    num_idxs_reg=(c1 - c0) * 128,
                                    elem_size=D, single_packet=False,
                                    queue_num=(q * 2 + s + i) % 4,
                                )
                        oh_t[q] = opool.tile([128, CHT, SSZ], bf16, tag=f"oh{q}", name=f"oh{q}")
                        nc.vector.tensor_tensor(
                            out=oh_t[q][:],
                            in0=iota_b[:].unsqueeze(1).broadcast_to([128, CHT, SSZ]),
                            in1=dstv_sb[:, t * CHT:(t + 1) * CHT]
                                .unsqueeze(2).broadcast_to([128, CHT, SSZ]),
                            op=mybir.AluOpType.is_equal,
                        )
                    agg = ppool.tile([128, SSZ], f32, tag="agg")
                    for q in (0, 1):
                        for k in range(CHT):
                            nc.tensor.matmul(
                                agg[:],
                                g_t[q][:, k, :],
                                oh_t[q][:, k, :],
                                start=(q == 0 and k == 0),
                                stop=(q == 1 and k == CHT - 1),
                                skip_group_check=True,
                            )
                    for pi in (0, 1):
                        pr = s * 2 + pi
                        agg_sb = spool.tile([128, 128], bf16, tag="aggsb")
                        nc.scalar.copy(agg_sb[:], agg[:, pi * 128:(pi + 1) * 128])
                        hps = hpool.tile([128, D], f32, tag="hps")
                        nc.tensor.matmul(hps[:], agg_sb[:], W_sb[:, l, :],
                                         start=True, stop=True)
                        t_sb = spool.tile([128, D], f32, tag="tsb")
                        nc.vector.scalar_tensor_tensor(
                            out=t_sb[:], in0=hps[:], scalar=ni_sb[:, pr:pr + 1],
                            in1=brep_sb[:, l, :],
                            op0=mybir.AluOpType.mult, op1=mybir.AluOpType.add,
                        )
                        if l < N_LAYERS - 1:
                            hn_bf = spool.tile([128, D], bf16, tag="hnb")
                            # relu(t)*no == relu(t*no) since no > 0; ACT is idle
                            nc.scalar.activation(
                                hn_bf[:], t_sb[:],
                                mybir.ActivationFunctionType.Relu,
                                scale=no_sb[:, pr:pr + 1],
                            )
                            r0 = (pr - H * 2 * NSUP_H) * 128
                            nc.sync.dma_start(
                                hn_half[l][H][r0:r0 + 128, :], hn_bf[:])
                        else:
                            h_bf = spool.tile([128, D], bf16, tag="hb")
                            nc.scalar.activation(
                                h_bf[:], t_sb[:],
                                mybir.ActivationFunctionType.Relu)
                            nc.tensor.matmul(
                                pool_ps[:], groh[:, pr, :], h_bf[:],
                                start=(pr == 0), stop=(pr == P - 1),
                            )
                    # fire the AllGather for half 0 as soon as it completes
                    if l < N_LAYERS - 1 and s == NSUP_H - 1 and not no_coll:
                        nc.gpsimd.collective_compute(
                            "AllGather", mybir.AluOpType.bypass,
                            replica_groups=[list(range(NCORES))],
                            ins=[hn_half[l][0][:].opt()],
                            outs=[ag_out[l][0].ap().opt()],
                        )
                if l < N_LAYERS - 1 and not no_coll:
                    nc.gpsimd.collective_compute(
                        "AllGather", mybir.AluOpType.bypass,
                        replica_groups=[list(range(NCORES))],
                        ins=[hn_half[l][1][:].opt()],
                        outs=[ag_out[l][1].ap().opt()],
                    )

            pool_sb = spool.tile([GPC, D], f32, tag="poolsb")
            nc.vector.tensor_scalar_mul(pool_sb[:], pool_ps[:], rc_sb[:])
            nc.sync.dma_start(out_t.ap(), pool_sb[:])

    nc.compile()
    return nc


def make_in_maps(per_core, shared):
    in_maps = []
    for c in range(NCORES):
        pc = per_core[c]
        in_maps.append({
            "idx": pc["idx"], "dstv": pc["dstv"], "t0exp": pc["t0exp"],
            "ni": pc["ni"], "no": pc["no"], "gid": pc["gid"], "rc": pc["rc"],
            "W": shared["W"], "b_rep": shared["b_rep"],
        })
    return in_maps


def kernel(**inputs) -> np.ndarray:
    per_core, shared, meta = preprocess(**inputs)
    nc = build(meta, rep=1)
    in_maps = make_in_maps(per_core, shared)
    res = run_bass_kernel_spmd(nc, in_maps, core_ids=list(range(NCORES)))
    return np.concatenate([res.results[c]["out"] for c in range(NCORES)], axis=0)
